# revision 16
# baseline (speedup 1.0000x reference)
"""BEV->Cylinder bilinear ring-sampling kernel for 8 Trainium2 NeuronCores.

Strategy (per core, 64 of the 512 (b,c) planes):
  * The 2048 sample points lie on a circle of radius 255.5 px; only a thin
    ring of the 512x512 BEV image is ever read. A host-precomputed rectangle
    cover of that ring is DMA'd into SBUF as X[plane, ring_px], split into
    two halves by sampling angle (p<1024 / p>=1024) living on SBUF
    partitions 0-63 / 64-127 at identical free offsets.
  * Each 128-px offset is PE-transposed for BOTH halves at once
    ([128,128] transpose) and copied to fp16 XT[px, (half,plane)].
  * col[(plane,zh), p] = sum_k w_k * I[corner_k(p)] via fp16 PE matmuls
    col[:, window] += XT_chunk(dup) @ S_chunk accumulated in PSUM[128,512]
    per 512-phi window; the stationary operand duplicates the 64 planes
    with a stride-0 AP so the output partition is plane*2+zh.
  * Per window: PSUM copyback to col_int, then one broadcast output DMA
    (z replication via stride-0 src AP) writes out[plane, z, window].
  * DMA issue is spread over Sync-HWDGE / Scalar-HWDGE (ring rects) and
    Pool-SWDGE (some rects + the 4 output DMAs) so descriptor generation
    never serializes behind one queue and output writes start as soon as
    window 0 is reduced.
All geometry/weights are input-independent compile-time constants baked into
the NEFF. Input dtype f32 is preserved end to end.
"""
import json
import math

import numpy as np

B, C, H_B, W_B = 4, 128, 512, 512
H_C, W_C = 64, 2048
MAX_RANGE = 50.0
XMIN, XMAX, YMIN, YMAX = -50.0, 50.0, -50.0, 50.0
NCORES = 8
PLANES = B * C // NCORES  # 64 planes per core

BAND = 16         # rows per cover band
CLUSTER_GAP = 16  # split x-clusters when gap exceeds this
MIN_W = 16        # min rect width (64B DMA bursts)
CHUNK = 128       # ring pixels per PE-transpose chunk
P_GAP = 128       # split p-interval when gap exceeds this
NQ = 4            # output column windows
QW = W_C // NQ    # 512 phi per window

_CACHE = {}


# ----------------------------------------------------------------- geometry
def _sample_xy():
    """Sampling pixel coords exactly like the reference (jnp on CPU);
    numpy fallback differs only at ULP level."""
    try:
        import jax
        import jax.numpy as jnp
        cpu = jax.devices("cpu")[0]
        with jax.default_device(cpu):
            phi = jnp.linspace(-math.pi, math.pi, W_C)
            x_g = MAX_RANGE * jnp.cos(phi)
            y_g = MAX_RANGE * jnp.sin(phi)
            x = (x_g - XMIN) / (XMAX - XMIN) * (W_B - 1)
            y = (YMAX - y_g) / (YMAX - YMIN) * (H_B - 1)
            return np.asarray(x, np.float32), np.asarray(y, np.float32)
    except Exception:
        phi = np.linspace(-math.pi, math.pi, W_C, dtype=np.float32)
        x_g = (MAX_RANGE * np.cos(phi)).astype(np.float32)
        y_g = (MAX_RANGE * np.sin(phi)).astype(np.float32)
        x = ((x_g - XMIN) / (XMAX - XMIN) * (W_B - 1)).astype(np.float32)
        y = ((YMAX - y_g) / (YMAX - YMIN) * (H_B - 1)).astype(np.float32)
        return x, y


def _corners():
    x, y = _sample_xy()
    x0 = np.floor(x).astype(np.int64)
    y0 = np.floor(y).astype(np.int64)
    wx1 = (x - x0.astype(np.float32)).astype(np.float32)
    wx0 = (np.float32(1.0) - wx1).astype(np.float32)
    wy1 = (y - y0.astype(np.float32)).astype(np.float32)
    wy0 = (np.float32(1.0) - wy1).astype(np.float32)
    out = []
    for xi, wx in ((x0, wx0), (x0 + 1, wx1)):
        for yi, wy in ((y0, wy0), (y0 + 1, wy1)):
            w = (wx * wy).astype(np.float32)
            valid = (xi >= 0) & (xi < W_B) & (yi >= 0) & (yi < H_B)
            for p in range(W_C):
                if valid[p]:
                    out.append((p, int(yi[p]), int(xi[p]), float(w[p])))
    return out


def _rect_p(rc):
    ya, h, xa, w = rc
    yc, xc = ya + h / 2.0, xa + w / 2.0
    phi = math.atan2(255.5 - yc, xc - 255.5)
    return (phi + math.pi) / (2 * math.pi)


def build_plan():
    corner_list = _corners()

    # rectangle cover of the ring, per BAND-row band
    need = {}
    row_of_band = {}
    for p, yy, xx, w in corner_list:
        need.setdefault(yy // BAND, set()).add(xx)
        row_of_band.setdefault(yy // BAND, set()).add(yy)
    rects = []
    for b in sorted(need):
        xs = sorted(need[b])
        rows = sorted(row_of_band[b])
        ya, yb = min(rows), max(rows) + 1
        cl_start = prev = xs[0]
        for xx in xs[1:] + [None]:
            if xx is None or xx - prev > CLUSTER_GAP:
                xa, xbb = cl_start, prev + 1
                if xbb - xa < MIN_W:
                    xbb = min(xa + MIN_W, W_B)
                    xa = xbb - MIN_W
                rects.append((ya, yb - ya, xa, xbb - xa))
                if xx is not None:
                    cl_start = xx
            if xx is not None:
                prev = xx
    # order along the sampling circle so downstream windows complete in p
    # order and output DMAs unblock early
    rects.sort(key=_rect_p)

    # split halves at p=0.5 (phi window boundary w1|w2) so that offset k of
    # half0 and half1 can be PE-transposed together ([128,128] both halves)
    split = sum(1 for rc in rects if _rect_p(rc) < 0.5)
    halves = [rects[:split], rects[split:]]
    rect_dmas = []   # (half, local_off, ya, h, xa, w)
    half_len = []    # padded length of each half
    half_last = []   # end of real data per half
    pix2ring = {}    # (y, x) -> (half, local ring position)
    for hh, rl in enumerate(halves):
        off = 0
        for (ya, h, xa, w) in rl:
            rect_dmas.append((hh, off, ya, h, xa, w))
            for r in range(h):
                for ccc in range(w):
                    key = (ya + r, xa + ccc)
                    if key not in pix2ring:
                        pix2ring[key] = (hh, off + r * w + ccc)
            off += h * w
        half_last.append(off)
        off += (-off) % CHUNK
        half_len.append(off)
    hl = max(half_len)
    nslots = hl // CHUNK

    # hits per (slot, half): p -> {r: weight}
    hits = {}
    for p, yy, xx, w in corner_list:
        hh, rp = pix2ring[(yy, xx)]
        d = hits.setdefault((rp // CHUNK, hh), {}).setdefault(p, {})
        r = rp % CHUNK
        d[r] = d.get(r, 0.0) + w

    # first window needing each (slot, half); transposes stay per-half so a
    # half0 chunk never waits on half1 ring data
    minw = {}
    for (k, hh), d in hits.items():
        minw[(k, hh)] = min(p // QW for p in d)
    tp_of_w = [[] for _ in range(NQ)]
    for kh in sorted(minw):
        tp_of_w[minw[kh]].append(kh)
    slot = {kh: i for i, kh in enumerate(
        kh for w in range(NQ) for kh in tp_of_w[w])}

    # matmuls: per (slot, half), intervals split at window boundaries/gaps
    mms_of_w = [[] for _ in range(NQ)]  # (k, hh, pst, plen, s_off)
    s_cols = []
    s_off = 0
    for (k, hh) in sorted(hits):
        d = hits[(k, hh)]
        ps = sorted(d)
        st = prev = ps[0]
        ivs = []
        for p in ps[1:] + [None]:
            if p is None or p - prev > P_GAP:
                ivs.append((st, prev - st + 1))
                if p is not None:
                    st = p
            if p is not None:
                prev = p
        split_ivs = []
        for (pst, plen) in ivs:
            a = pst
            while a < pst + plen:
                bq = min(pst + plen, (a // QW + 1) * QW)
                split_ivs.append((a, bq - a))
                a = bq
        for (pst, plen) in split_ivs:
            mms_of_w[pst // QW].append((k, hh, pst, plen, s_off))
            for p in range(pst, pst + plen):
                s_cols.append((d.get(p, {}),))
            s_off += plen
    S = np.zeros((CHUNK, s_off), dtype=np.float32)
    for j, (rows,) in enumerate(s_cols):
        for r, w in rows.items():
            S[r, j] += np.float32(w)

    # fp16-shadow converts grouped by the first window whose transposes read
    # any chunk overlapping the rect (program-order read-before-write safety)
    cv_of_w = [[] for _ in range(NQ)]
    for ri, (hh, off, ya, h, xa, w) in enumerate(rect_dmas):
        wq = NQ - 1
        for k in range(off // CHUNK, (off + h * w - 1) // CHUNK + 1):
            if (k, hh) in minw:
                wq = min(wq, minw[(k, hh)])
        cv_of_w[wq].append(ri)

    return dict(rect_dmas=rect_dmas, half_len=half_len, half_last=half_last,
                hl=hl, nslots=nslots, S=S.astype(np.float16), sum_m=s_off,
                tp_of_w=tp_of_w, mms_of_w=mms_of_w, slot=slot,
                cv_of_w=cv_of_w)


# ------------------------------------------------------- walrus wait-split
def split_waits_json(bir, maxw=1):
    """This neuronxcc walrus accepts at most one sync-wait per instruction;
    move excess waits onto preceding wait-only EventSemaphore ops."""
    uid = [0]
    for fn in bir["functions"]:
        for blk in fn["blocks"]:
            out = []
            for inst in blk["instructions"]:
                si = inst.get("sync_info")
                if si and si.get("on_wait") and len(si["on_wait"]) > maxw:
                    waits = si["on_wait"]
                    extra, keep = waits[:-maxw], waits[-maxw:]
                    for i in range(0, len(extra), maxw):
                        uid[0] += 1
                        out.append({
                            "debug": inst.get("debug", 0),
                            "engine": inst["engine"],
                            "ins": [],
                            "name": f"I-ws-{uid[0]}",
                            "opcode": "EventSemaphore",
                            "outs": [],
                            "sync_info": {"on_update": [],
                                          "on_wait": extra[i:i + maxw]},
                        })
                    si["on_wait"] = keep
                out.append(inst)
            blk["instructions"] = out
    return bir


# ------------------------------------------------------------ device build
def build_nc(plan, repeat=1):
    import concourse.bass as bass
    import concourse.mybir as mybir
    from concourse.tile import TileContext

    class PatchedBass(bass.Bass):
        def to_json_bytes(self):
            data = json.loads(super().to_json_bytes())
            return json.dumps(split_waits_json(data, 1)).encode()

    nc = PatchedBass()
    x_in = nc.dram_tensor("bev", [PLANES, H_B, W_B], mybir.dt.float32,
                          kind="ExternalInput")
    # unused input whose shape varies with `repeat`: defeats the NEFF cache's
    # shape-only HLO hash so timing variants compile separately
    nc.dram_tensor("nonce", [1, max(1, repeat)], mybir.dt.float32,
                   kind="ExternalInput")
    out = nc.dram_tensor("out", [PLANES, H_C, W_C], mybir.dt.float32,
                         kind="ExternalOutput")
    s_const = nc.inline_tensor(plan["S"], name="s_const")
    ident = nc.inline_tensor(np.tile(np.eye(64, dtype=np.float16), (2, 1)),
                             name="ident")

    hl = plan["hl"]
    nkh = len(plan["slot"])

    with TileContext(nc) as tc:
        with tc.tile_pool(name="sb", bufs=1) as pool, \
             tc.tile_pool(name="pst", bufs=4, space="PSUM") as pst, \
             tc.tile_pool(name="psc", bufs=1, space="PSUM") as psc:
            x_sb = pool.tile([128, hl], mybir.dt.float32)
            xb_sb = pool.tile([128, hl], mybir.dt.float16)
            xt_sb = pool.tile([128, nkh * 2 * 64], mybir.dt.float16)
            s_sb = pool.tile([CHUNK, plan["sum_m"]], mybir.dt.float16)
            id_sb = pool.tile([128, 64], mybir.dt.float16)
            zero_sb = pool.tile([128, 1], mybir.dt.float16)
            col_int = pool.tile([128, W_C], mybir.dt.float32)

            nc.gpsimd.dma_start(s_sb[:], s_const[:])
            nc.gpsimd.dma_start(id_sb[:], ident[:])
            nc.gpsimd.memset(zero_sb[:], 0.0)

            for _rep in range(repeat):
              # pad regions of the fp16 shadow must read as zeros
              for hh in (0, 1):
                  last = plan["half_last"][hh]
                  if hl > last:
                      nc.gpsimd.memset(xb_sb[64 * hh:64 * hh + 64, last:hl],
                                       0.0)

              # ring cover loads; Pool SWDGE takes half the rects (its
              # descriptor generation is ~2x faster per rect), the HWDGE
              # queues on Sync/Scalar split the rest. p-sorted so early
              # windows land first.
              for ri, (hh, off, ya, h, xa, w) in enumerate(plan["rect_dmas"]):
                  eng = (nc.gpsimd, nc.sync, nc.gpsimd, nc.scalar)[ri % 4]
                  eng.dma_start(
                      x_sb[64 * hh:64 * hh + 64, off:off + h * w],
                      x_in[:, ya:ya + h, xa:xa + w])

              # init the 4 window PSUM tiles to zero (K=1 zero matmul)
              col_tiles = []
              for q in range(NQ):
                  col_q = psc.tile([128, QW], mybir.dt.float32,
                                   name=f"colq{q}", tag=f"colq{q}")
                  col_tiles.append(col_q)
                  nc.tensor.matmul(
                      col_q[:],
                      zero_sb[:1, :1].to_broadcast((1, 128)),
                      zero_sb[:1, :1].to_broadcast((1, QW)),
                      start=True, stop=False, skip_group_check=True)

              # pipelined: per window, converts -> transposes -> matmuls ->
              # copyback -> broadcast output DMA
              for q in range(NQ):
                  for ri in plan["cv_of_w"][q]:
                      hh, off, ya, h, w_ = (plan["rect_dmas"][ri][0],
                                            plan["rect_dmas"][ri][1],
                                            plan["rect_dmas"][ri][2],
                                            plan["rect_dmas"][ri][3],
                                            plan["rect_dmas"][ri][5])
                      ph = slice(64 * hh, 64 * hh + 64)
                      rs = slice(off, off + h * w_)
                      # f32 -> fp16 shadow per rect; windowed so this
                      # window's dups don't queue behind later converts
                      nc.vector.tensor_copy(xb_sb[ph, rs], x_sb[ph, rs])
                  for (k, hh) in plan["tp_of_w"][q]:
                      sl = plan["slot"][(k, hh)]
                      ph = slice(64 * hh, 64 * hh + 64)
                      cs = slice(k * CHUNK, (k + 1) * CHUNK)
                      xt_ps = pst.tile([128, 64], mybir.dt.float16,
                                       name=f"xtps{q}_{k}_{hh}", tag="xtps")
                      nc.tensor.transpose(xt_ps[:], xb_sb[ph, cs],
                                          id_sb[ph, :])
                      # plane-major x2 dup (col j = src[j//2]) so the matmul
                      # stationary is a plain [128,128] slice whose output
                      # partition is plane*2+zh
                      nc.vector.tensor_copy(
                          xt_sb[:, sl * 128:(sl + 1) * 128],
                          xt_ps[:, :, None].to_broadcast((128, 64, 2)))
                  mms = plan["mms_of_w"][q]
                  nmq = len(mms)
                  col_q = col_tiles[q]
                  for i, (k, hh, pst_, plen, so) in enumerate(mms):
                      sl = plan["slot"][(k, hh)]
                      nc.tensor.matmul(
                          col_q[:, pst_ - QW * q:pst_ - QW * q + plen],
                          xt_sb[:, sl * 128:(sl + 1) * 128],
                          s_sb[:, so:so + plen],
                          start=False, stop=(i == nmq - 1),
                          skip_group_check=True)
                  qs = slice(QW * q, QW * (q + 1))
                  cp = nc.vector.tensor_copy if q % 2 == 0 else nc.scalar.copy
                  cp(col_int[:, qs], col_q[:])
                  # broadcast output: partition = plane*2+zh, then z, w
                  srcb = col_int[:, None, qs].to_broadcast(
                      (128, H_C // 2, QW))
                  dst = bass.AP(out, QW * q, [
                      [H_C // 2 * W_C, 128],
                      [W_C, H_C // 2],
                      [1, QW],
                  ])
                  oeng = nc.sync if q % 2 == 0 else nc.scalar
                  oeng.dma_start(dst, srcb)
    return nc


# ------------------------------------------------------------------ runner
def _get_state():
    if "state" in _CACHE:
        return _CACHE["state"]
    import jax
    import concourse.mybir as mybir
    from concourse import bass2jax
    from jax.sharding import Mesh, PartitionSpec
    from jax.experimental.shard_map import shard_map

    plan = build_plan()
    nc = build_nc(plan)
    bass2jax.install_neuronx_cc_hook()

    partition_name = (nc.partition_id_tensor.name
                      if nc.partition_id_tensor else None)
    in_names, out_names, out_avals, zero_outs = [], [], [], []
    for alloc in nc.m.functions[0].allocations:
        if not isinstance(alloc, mybir.MemoryLocationSet):
            continue
        name = alloc.memorylocations[0].name
        if alloc.kind == "ExternalInput":
            if name != partition_name:
                in_names.append(name)
        elif alloc.kind == "ExternalOutput":
            shape = tuple(alloc.tensor_shape)
            dtype = mybir.dt.np(alloc.dtype)
            out_names.append(name)
            out_avals.append(jax.core.ShapedArray(shape, dtype))
            zero_outs.append(np.zeros(shape, dtype))
    n_params = len(in_names)
    n_outs = len(out_names)
    all_names = in_names + out_names
    if partition_name is not None:
        all_names = all_names + [partition_name]
    donate = tuple(range(n_params, n_params + n_outs))

    def _body(*args):
        operands = list(args)
        if partition_name is not None:
            operands.append(bass2jax.partition_id_tensor())
        outs = bass2jax._bass_exec_p.bind(
            *operands,
            out_avals=tuple(out_avals),
            in_names=tuple(all_names),
            out_names=tuple(out_names),
            lowering_input_output_aliases=(),
            sim_require_finite=True,
            sim_require_nnan=True,
            nc=nc,
        )
        return tuple(outs)

    devices = jax.devices()[:NCORES]
    mesh = Mesh(np.asarray(devices), ("core",))
    specs = (PartitionSpec("core"),) * (n_params + n_outs)
    out_specs = (PartitionSpec("core"),) * n_outs
    fn = jax.jit(
        shard_map(_body, mesh=mesh, in_specs=specs, out_specs=out_specs,
                  check_rep=False),
        donate_argnums=donate, keep_unused=True)

    nonce = np.zeros((NCORES, 1), np.float32)
    state = dict(fn=fn, zero_outs=zero_outs, prev=None, nc=nc, plan=plan,
                 nonce=nonce)
    _CACHE["state"] = state
    return state


def kernel(bev_feat):
    bev = np.ascontiguousarray(np.asarray(bev_feat, dtype=np.float32))
    st = _get_state()
    global_in = bev.reshape(B * C, H_B, W_B)  # cores split axis 0: 64 each
    if st["prev"] is not None:
        zouts = st["prev"]          # donate previous device outputs
    else:
        zouts = [np.zeros((NCORES * z.shape[0], *z.shape[1:]), z.dtype)
                 for z in st["zero_outs"]]
    outs = st["fn"](global_in, st["nonce"], *zouts)
    result = np.asarray(outs[0])    # [512, 64, 2048]
    st["prev"] = list(outs)
    return result.reshape(B, C, H_C, W_C)
